# revision 18
# baseline (speedup 1.0000x reference)
"""Trainium2 Bass kernel for nn_LoraLinear (DoRA-style LoraLinear forward).

Reference computation:
    Wc   = weight + 2.0 * (lora_B @ lora_A)            # [OUT, IN]
    norm = ||Wc||_2 over OUT axis + 1e-6               # [1, IN]
    out  = x @ (lora_M * Wc / norm).T + bias           # [B, S, OUT]

Key algebraic identity: (lora_M * Wc / norm).T applied to x equals
(x * s) @ Wc.T with s = lora_M / norm, so W_eff is never materialized.

Sharding (8 NeuronCores, tensor/column parallel):
    - OUT = 11008 = 8 * 1376 -> each core owns 1376 output columns of
      weight / lora_B / bias.
    - x, lora_A, lora_M replicated.
    - Each core computes partial column sums-of-squares of Wc over its
      OUT shard; two AllReduces (one per half of the IN dimension)
      combine the 8 partials, so the first collective and its dependent
      matmuls overlap the tail of the weight stream.

Device layout: IN lives on partitions everywhere. The host passes W and
x pre-transposed so tiles land as [IN-part, OUT-free] / [IN-part, tok],
which is what both the TensorEngine matmuls and the free-dim norm
reduction want. W streams in fp32 (norms in full precision); the
resident combined weight + activations are fp16 for full-rate matmuls.
"""

import functools

import numpy as np

import concourse.tile as tile
from concourse import bacc, mybir
from concourse.bass_utils import run_bass_kernel_spmd

F32 = mybir.dt.float32
F16 = mybir.dt.float16

NCORES = 8
B, S, IN, OUT, R = 8, 32, 4096, 11008, 64
TOK = B * S                      # 256
OSH = OUT // NCORES              # 1376 per-core output shard
NCHUNK = IN // 128               # 32 IN chunks of 128
N_TILES = (512, 512, 352)        # OUT-shard tiling for psum (<=512 fp32)
N_OFFS = (0, 512, 1024)
SCALING = 2.0
EPS = 1e-6
HALVES = (tuple(range(0, 12)), tuple(range(12, 32)))
N_WARM = 16


@functools.lru_cache(maxsize=1)
def _build():
    nc = bacc.Bacc("TRN2", target_bir_lowering=False, debug=False,
                   num_devices=NCORES)

    wt = nc.dram_tensor("wt", [IN, OSH], F32, kind="ExternalInput").ap()
    xt = nc.dram_tensor("xt", [IN, TOK], F16, kind="ExternalInput").ap()
    ap_ = nc.dram_tensor("ap", [128, IN // 2], F16, kind="ExternalInput").ap()
    b2t = nc.dram_tensor("b2t", [2 * R, OSH], F16, kind="ExternalInput").ap()
    bias = nc.dram_tensor("bias", [1, OSH], F16, kind="ExternalInput").ap()
    ones = nc.dram_tensor("ones", [1, 128], F16, kind="ExternalInput").ap()
    mt = nc.dram_tensor("mt", [128, NCHUNK], F32, kind="ExternalInput").ap()

    out = nc.dram_tensor("out", [TOK, OSH], F32, kind="ExternalOutput").ap()

    with tile.TileContext(nc) as tc:
        with (
            tc.tile_pool(name="wstream", bufs=8) as wsp,
            tc.tile_pool(name="wc", bufs=1) as wcp,
            tc.tile_pool(name="sb", bufs=1) as sb,
            tc.tile_pool(name="stage", bufs=2) as stp,
            tc.tile_pool(name="ps", bufs=2, space="PSUM") as psp,
            tc.tile_pool(name="pso", bufs=1, space="PSUM") as pso,
            tc.tile_pool(name="dram", bufs=1, space="DRAM") as dram,
        ):
            # ---- constants / small tensors ----
            a_sb = sb.tile([128, IN // 2], F16, name="a_sb")
            b2t_sb = sb.tile([2 * R, OSH], F16, name="b2t_sb")
            bias_sb = sb.tile([1, OSH], F16, name="bias_sb")
            ones_sb = sb.tile([1, 128], F16, name="ones_sb")
            m_sb = sb.tile([128, NCHUNK], F32, name="m_sb")
            x_all = sb.tile([128, NCHUNK * TOK], F16, name="x_all")
            warm_src = sb.tile([128, 512], F32, name="warm_src")
            nc.vector.memset(warm_src[:], 0.001)
            sq_scr = sb.tile([128, OSH], F16, name="sq_scr")

            nc.sync.dma_start(a_sb[:], ap_)
            nc.sync.dma_start(b2t_sb[:], b2t)
            nc.sync.dma_start(bias_sb[:], bias)
            nc.sync.dma_start(ones_sb[:], ones)
            nc.sync.dma_start(m_sb[:], mt)

            # ---- persistent psum accumulators, seeded with the bias ----
            psum_out = {}
            for m in range(2):
                for n in range(3):
                    psum_out[m, n] = pso.tile(
                        [128, N_TILES[n]], F32, name=f"po{m}{n}", tag=f"po{m}{n}"
                    )
                    nc.tensor.matmul(
                        psum_out[m, n][:],
                        ones_sb[0:1, 0:128],
                        bias_sb[0:1, N_OFFS[n]:N_OFFS[n] + N_TILES[n]],
                        start=True, stop=False,
                    )

            wc_tiles = {}
            sq_insts = []

            def phase1(c, parts, jcol):
                """Stream W chunk c, build resident fp16 Wc, norm partial."""
                wst = wsp.tile([128, OSH], F32, name=f"ws{c}", tag="ws")
                nc.sync.dma_start(wst[:], wt[c * 128:(c + 1) * 128, :])
                wct = wcp.tile([128, OSH], F16, name=f"wc{c}", tag=f"wc{c}")
                wc_tiles[c] = wct
                if c < NCHUNK // 2:
                    a_chunk = a_sb[0:64, c * 128:(c + 1) * 128]
                    rlo = 0
                else:
                    cc = c - NCHUNK // 2
                    a_chunk = a_sb[64:128, cc * 128:(cc + 1) * 128]
                    rlo = 64
                for n in range(3):
                    nsl = slice(N_OFFS[n], N_OFFS[n] + N_TILES[n])
                    pl = psp.tile([128, 512], F32, name=f"pl{c}{n}", tag="pl")
                    nc.tensor.matmul(
                        pl[:, 0:N_TILES[n]], a_chunk,
                        b2t_sb[rlo:rlo + 64, nsl],
                        start=True, stop=True,
                    )
                    # Wc(fp16) = W(fp32) + 2*B@A(psum fp32)
                    nc.vector.tensor_tensor(
                        out=wct[:, nsl], in0=wst[:, nsl],
                        in1=pl[:, 0:N_TILES[n]], op=mybir.AluOpType.add,
                    )
                # one wide column-sum-of-squares per chunk on ACT; the
                # squared values go to a scratch tile and are discarded
                sq_insts.append(nc.scalar.activation(
                    sq_scr[:], wct[:],
                    mybir.ActivationFunctionType.Square,
                    accum_out=parts[:, jcol:jcol + 1],
                ))
                # prefetch x chunk (fp16) into the big resident x buffer
                # (scalar HWDGE ring, so the sync ring stays pure W stream)
                nc.scalar.dma_start(
                    x_all[:, c * TOK:(c + 1) * TOK],
                    xt[c * 128:(c + 1) * 128, :],
                )

            def phase23(h, chunks, parts):
                """AllReduce this half's partials, compute s, warm the PE,
                then run this half's output matmuls."""
                nch = len(chunks)
                lo = chunks[0]
                cin = dram.tile([128, nch], F32, name=f"ci{h}", tag=f"ci{h}")
                cout = dram.tile([128, nch], F32, name=f"co{h}", tag=f"co{h}",
                                 addr_space="Shared")
                nc.gpsimd.dma_start(cin[:], parts[:])
                nc.gpsimd.collective_compute(
                    "AllReduce", mybir.AluOpType.add,
                    replica_groups=[list(range(NCORES))],
                    ins=[cin[:].opt()], outs=[cout[:].opt()],
                )
                n2g = sb.tile([128, nch], F32, name=f"n2g{h}", tag=f"n2g{h}")
                nc.sync.dma_start(n2g[:], cout[:])

                # PE warmup: junk matmuls gated on this half's partials so
                # the PE clock stays hot across the collective window
                wps = stp.tile([16, 32], F32, name=f"wps{h}", tag="wps")
                for wi in range(N_WARM):
                    plw = psp.tile([128, 512], F32, name=f"plw{h}{wi}",
                                   tag="pl")
                    nc.tensor.matmul(plw[0:nch, :], parts[:],
                                     warm_src[:],
                                     start=True, stop=True)
                    if wi == N_WARM - 1:
                        nc.vector.tensor_copy(wps[:], plw[0:16, 0:32])

                # s = m/(sqrt(n2g)+eps): ACT sqrt + Newton, DVE recip + Newton
                y0 = sb.tile([128, nch], F32, name=f"y0{h}", tag=f"y0{h}")
                sqrt_inst = nc.scalar.activation(
                    y0[:], n2g[:], mybir.ActivationFunctionType.Sqrt)
                from concourse.bass import _add_dep_helper
                _add_dep_helper(sqrt_inst.ins, sq_insts[-1].ins, False,
                                "keep ACT FIFO clear of the sqrt wait")
                r0 = sb.tile([128, nch], F32, name=f"r0{h}", tag=f"r0{h}")
                nc.vector.reciprocal(r0[:], y0[:])
                t0 = sb.tile([128, nch], F32, name=f"t0{h}", tag=f"t0{h}")
                nc.vector.tensor_tensor(out=t0[:], in0=n2g[:], in1=r0[:],
                                        op=mybir.AluOpType.mult)
                y1 = sb.tile([128, nch], F32, name=f"y1{h}", tag=f"y1{h}")
                nc.vector.tensor_tensor(out=y1[:], in0=y0[:], in1=t0[:],
                                        op=mybir.AluOpType.add)
                nc.vector.tensor_scalar(out=y1[:], in0=y1[:], scalar1=0.5,
                                        scalar2=EPS, op0=mybir.AluOpType.mult,
                                        op1=mybir.AluOpType.add)
                r1 = sb.tile([128, nch], F32, name=f"r1{h}", tag=f"r1{h}")
                nc.vector.reciprocal(r1[:], y1[:])
                t2 = sb.tile([128, nch], F32, name=f"t2{h}", tag=f"t2{h}")
                nc.vector.tensor_tensor(out=t2[:], in0=y1[:], in1=r1[:],
                                        op=mybir.AluOpType.mult)
                u = sb.tile([128, nch], F32, name=f"u{h}", tag=f"u{h}")
                nc.vector.tensor_scalar(out=u[:], in0=t2[:], scalar1=-1.0,
                                        scalar2=2.0, op0=mybir.AluOpType.mult,
                                        op1=mybir.AluOpType.add)
                r2 = sb.tile([128, nch], F32, name=f"r2{h}", tag=f"r2{h}")
                nc.vector.tensor_tensor(out=r2[:], in0=r1[:], in1=u[:],
                                        op=mybir.AluOpType.mult)
                s_sb = sb.tile([128, nch], F32, name=f"s{h}", tag=f"s{h}")
                nc.vector.tensor_tensor(out=s_sb[:],
                                        in0=m_sb[:, lo:lo + nch],
                                        in1=r2[:], op=mybir.AluOpType.mult)

                # xs = x * s (in place), then this half's output matmuls
                for j, c in enumerate(chunks):
                    xsl = slice(c * TOK, (c + 1) * TOK)
                    nc.vector.tensor_scalar_mul(
                        x_all[:, xsl], x_all[:, xsl], s_sb[:, j:j + 1])
                if h < len(HALVES) - 1:
                    for c in chunks:
                        for m in range(2):
                            lhs = x_all[:, c * TOK + m * 128:
                                        c * TOK + (m + 1) * 128]
                            for n in range(3):
                                nsl = slice(N_OFFS[n],
                                            N_OFFS[n] + N_TILES[n])
                                nc.tensor.matmul(
                                    psum_out[m, n][:], lhs,
                                    wc_tiles[c][:, nsl],
                                    start=False, stop=False,
                                )
                else:
                    # n-outer so each psum accumulator finishes early and
                    # its drain overlaps the remaining matmuls
                    for n in range(3):
                        nsl = slice(N_OFFS[n], N_OFFS[n] + N_TILES[n])
                        for c in chunks:
                            last = c == NCHUNK - 1
                            for m in range(2):
                                lhs = x_all[:, c * TOK + m * 128:
                                            c * TOK + (m + 1) * 128]
                                nc.tensor.matmul(
                                    psum_out[m, n][:], lhs,
                                    wc_tiles[c][:, nsl],
                                    start=False, stop=last,
                                )
                        for m in range(2):
                            st = stp.tile([128, N_TILES[n]], F32,
                                          name=f"st{m}{n}", tag="st")
                            nc.vector.tensor_copy(st[:], psum_out[m, n][:])
                            nc.sync.dma_start(
                                out[m * 128:(m + 1) * 128, nsl], st[:])

            parts_h = {}
            for h, chunks in enumerate(HALVES):
                parts_h[h] = sb.tile([128, len(chunks)], F32,
                                     name=f"parts{h}", tag=f"parts{h}")
                for j, c in enumerate(chunks):
                    phase1(c, parts_h[h], j)
            for h, chunks in enumerate(HALVES):
                phase23(h, chunks, parts_h[h])

    nc.compile()
    return nc


def _prep_inputs(x, weight, lora_A, lora_B, lora_M, bias):
    """Shard + lay out the full inputs for the 8 cores (host-side data
    marshaling only)."""
    x = np.asarray(x, np.float32)
    weight = np.asarray(weight, np.float32)
    lora_A = np.asarray(lora_A, np.float32)
    lora_B = np.asarray(lora_B, np.float32)
    lora_M = np.asarray(lora_M, np.float32)
    bias = np.asarray(bias, np.float32)

    xt = np.ascontiguousarray(x.reshape(TOK, IN).T.astype(np.float16))
    a_packed = np.empty((128, IN // 2), np.float16)
    a_packed[0:64] = lora_A[:, 0:IN // 2]
    a_packed[64:128] = lora_A[:, IN // 2:]
    mt = np.ascontiguousarray(lora_M.reshape(NCHUNK, 128).T)     # [128, 32]
    ones = np.ones((1, 128), np.float16)

    in_maps = []
    for c in range(NCORES):
        sl = slice(c * OSH, (c + 1) * OSH)
        b2t_half = (SCALING * lora_B[sl, :]).T.astype(np.float16)
        in_maps.append(dict(
            wt=np.ascontiguousarray(weight[sl, :].T),            # [IN, OSH] f32
            xt=xt,
            ap=a_packed,
            b2t=np.ascontiguousarray(np.concatenate([b2t_half] * 2, axis=0)),
            bias=np.ascontiguousarray(
                bias[sl].reshape(1, OSH).astype(np.float16)),
            ones=ones,
            mt=mt,
        ))
    return in_maps


def _run(inputs, trace=False):
    nc = _build()
    in_maps = _prep_inputs(**inputs)
    res = run_bass_kernel_spmd(nc, in_maps, core_ids=list(range(NCORES)),
                               trace=trace)
    full = np.concatenate([res.results[c]["out"] for c in range(NCORES)],
                          axis=1)
    return full.reshape(B, S, OUT), res


def kernel(x, weight, lora_A, lora_B, lora_M, bias):
    out, _ = _run(dict(x=x, weight=weight, lora_A=lora_A, lora_B=lora_B,
                       lora_M=lora_M, bias=bias))
    return out


def kernel_profiled(**inputs):
    """Like kernel() but with NTFF tracing; returns (out, exec_time_ns)."""
    out, res = _run(inputs, trace=True)
    return out, res.exec_time_ns


# revision 19
# speedup vs baseline: 1.0005x; 1.0005x over previous
"""Trainium2 Bass kernel for nn_LoraLinear (DoRA-style LoraLinear forward).

Reference computation:
    Wc   = weight + 2.0 * (lora_B @ lora_A)            # [OUT, IN]
    norm = ||Wc||_2 over OUT axis + 1e-6               # [1, IN]
    out  = x @ (lora_M * Wc / norm).T + bias           # [B, S, OUT]

Key algebraic identity: (lora_M * Wc / norm).T applied to x equals
(x * s) @ Wc.T with s = lora_M / norm, so W_eff is never materialized.

Sharding (8 NeuronCores, tensor/column parallel):
    - OUT = 11008 = 8 * 1376 -> each core owns 1376 output columns of
      weight / lora_B / bias.
    - x, lora_A, lora_M replicated.
    - Each core computes partial column sums-of-squares of Wc over its
      OUT shard; two AllReduces (one per half of the IN dimension)
      combine the 8 partials, so the first collective and its dependent
      matmuls overlap the tail of the weight stream.

Device layout: IN lives on partitions everywhere. The host passes W and
x pre-transposed so tiles land as [IN-part, OUT-free] / [IN-part, tok],
which is what both the TensorEngine matmuls and the free-dim norm
reduction want. W streams in fp32 (norms in full precision); the
resident combined weight + activations are fp16 for full-rate matmuls.
"""

import functools

import numpy as np

import concourse.tile as tile
from concourse import bacc, mybir
from concourse.bass_utils import run_bass_kernel_spmd

F32 = mybir.dt.float32
F16 = mybir.dt.float16

NCORES = 8
B, S, IN, OUT, R = 8, 32, 4096, 11008, 64
TOK = B * S                      # 256
OSH = OUT // NCORES              # 1376 per-core output shard
NCHUNK = IN // 128               # 32 IN chunks of 128
N_TILES = (512, 512, 352)        # OUT-shard tiling for psum (<=512 fp32)
N_OFFS = (0, 512, 1024)
SCALING = 2.0
EPS = 1e-6
HALVES = (tuple(range(0, 16)), tuple(range(16, 32)))
N_WARM = 16


@functools.lru_cache(maxsize=1)
def _build():
    nc = bacc.Bacc("TRN2", target_bir_lowering=False, debug=False,
                   num_devices=NCORES)

    wt = nc.dram_tensor("wt", [IN, OSH], F32, kind="ExternalInput").ap()
    xt = nc.dram_tensor("xt", [IN, TOK], F16, kind="ExternalInput").ap()
    ap_ = nc.dram_tensor("ap", [128, IN // 2], F16, kind="ExternalInput").ap()
    b2t = nc.dram_tensor("b2t", [2 * R, OSH], F16, kind="ExternalInput").ap()
    bias = nc.dram_tensor("bias", [1, OSH], F16, kind="ExternalInput").ap()
    ones = nc.dram_tensor("ones", [1, 128], F16, kind="ExternalInput").ap()
    mt = nc.dram_tensor("mt", [128, NCHUNK], F32, kind="ExternalInput").ap()

    out = nc.dram_tensor("out", [TOK, OSH], F32, kind="ExternalOutput").ap()

    with tile.TileContext(nc) as tc:
        with (
            tc.tile_pool(name="wstream", bufs=8) as wsp,
            tc.tile_pool(name="wc", bufs=1) as wcp,
            tc.tile_pool(name="sb", bufs=1) as sb,
            tc.tile_pool(name="stage", bufs=2) as stp,
            tc.tile_pool(name="ps", bufs=2, space="PSUM") as psp,
            tc.tile_pool(name="pso", bufs=1, space="PSUM") as pso,
            tc.tile_pool(name="dram", bufs=1, space="DRAM") as dram,
        ):
            # ---- constants / small tensors ----
            a_sb = sb.tile([128, IN // 2], F16, name="a_sb")
            b2t_sb = sb.tile([2 * R, OSH], F16, name="b2t_sb")
            bias_sb = sb.tile([1, OSH], F16, name="bias_sb")
            ones_sb = sb.tile([1, 128], F16, name="ones_sb")
            m_sb = sb.tile([128, NCHUNK], F32, name="m_sb")
            x_all = sb.tile([128, NCHUNK * TOK], F16, name="x_all")
            warm_src = sb.tile([128, 512], F32, name="warm_src")
            nc.vector.memset(warm_src[:], 0.001)
            sq_scr = sb.tile([128, OSH], F16, name="sq_scr")

            nc.sync.dma_start(a_sb[:], ap_)
            nc.sync.dma_start(b2t_sb[:], b2t)
            nc.sync.dma_start(bias_sb[:], bias)
            nc.sync.dma_start(ones_sb[:], ones)
            nc.sync.dma_start(m_sb[:], mt)

            # ---- persistent psum accumulators, seeded with the bias ----
            psum_out = {}
            for m in range(2):
                for n in range(3):
                    psum_out[m, n] = pso.tile(
                        [128, N_TILES[n]], F32, name=f"po{m}{n}", tag=f"po{m}{n}"
                    )
                    nc.tensor.matmul(
                        psum_out[m, n][:],
                        ones_sb[0:1, 0:128],
                        bias_sb[0:1, N_OFFS[n]:N_OFFS[n] + N_TILES[n]],
                        start=True, stop=False,
                    )

            wc_tiles = {}
            sq_insts = []

            def phase1(c, parts, jcol):
                """Stream W chunk c, build resident fp16 Wc, norm partial."""
                wst = wsp.tile([128, OSH], F32, name=f"ws{c}", tag="ws")
                nc.sync.dma_start(wst[:], wt[c * 128:(c + 1) * 128, :])
                wct = wcp.tile([128, OSH], F16, name=f"wc{c}", tag=f"wc{c}")
                wc_tiles[c] = wct
                if c < NCHUNK // 2:
                    a_chunk = a_sb[0:64, c * 128:(c + 1) * 128]
                    rlo = 0
                else:
                    cc = c - NCHUNK // 2
                    a_chunk = a_sb[64:128, cc * 128:(cc + 1) * 128]
                    rlo = 64
                for n in range(3):
                    nsl = slice(N_OFFS[n], N_OFFS[n] + N_TILES[n])
                    pl = psp.tile([128, 512], F32, name=f"pl{c}{n}", tag="pl")
                    nc.tensor.matmul(
                        pl[:, 0:N_TILES[n]], a_chunk,
                        b2t_sb[rlo:rlo + 64, nsl],
                        start=True, stop=True,
                    )
                    # Wc(fp16) = W(fp32) + 2*B@A(psum fp32)
                    nc.vector.tensor_tensor(
                        out=wct[:, nsl], in0=wst[:, nsl],
                        in1=pl[:, 0:N_TILES[n]], op=mybir.AluOpType.add,
                    )
                # one wide column-sum-of-squares per chunk on ACT; the
                # squared values go to a scratch tile and are discarded
                sq_insts.append(nc.scalar.activation(
                    sq_scr[:], wct[:],
                    mybir.ActivationFunctionType.Square,
                    accum_out=parts[:, jcol:jcol + 1],
                ))
                # prefetch x chunk (fp16) into the big resident x buffer
                nc.sync.dma_start(
                    x_all[:, c * TOK:(c + 1) * TOK],
                    xt[c * 128:(c + 1) * 128, :],
                )

            def phase23(h, chunks, parts):
                """AllReduce this half's partials, compute s, warm the PE,
                then run this half's output matmuls."""
                nch = len(chunks)
                lo = chunks[0]
                cin = dram.tile([128, nch], F32, name=f"ci{h}", tag=f"ci{h}")
                cout = dram.tile([128, nch], F32, name=f"co{h}", tag=f"co{h}",
                                 addr_space="Shared")
                nc.gpsimd.dma_start(cin[:], parts[:])
                nc.gpsimd.collective_compute(
                    "AllReduce", mybir.AluOpType.add,
                    replica_groups=[list(range(NCORES))],
                    ins=[cin[:].opt()], outs=[cout[:].opt()],
                )
                n2g = sb.tile([128, nch], F32, name=f"n2g{h}", tag=f"n2g{h}")
                nc.sync.dma_start(n2g[:], cout[:])

                # PE warmup: junk matmuls gated on this half's partials so
                # the PE clock stays hot across the collective window
                wps = stp.tile([16, 32], F32, name=f"wps{h}", tag="wps")
                for wi in range(N_WARM):
                    plw = psp.tile([128, 512], F32, name=f"plw{h}{wi}",
                                   tag="pl")
                    nc.tensor.matmul(plw[0:nch, :], parts[:],
                                     warm_src[:],
                                     start=True, stop=True)
                    if wi == N_WARM - 1:
                        nc.vector.tensor_copy(wps[:], plw[0:16, 0:32])

                # s = m/(sqrt(n2g)+eps): ACT sqrt + Newton, DVE recip + Newton
                y0 = sb.tile([128, nch], F32, name=f"y0{h}", tag=f"y0{h}")
                sqrt_inst = nc.scalar.activation(
                    y0[:], n2g[:], mybir.ActivationFunctionType.Sqrt)
                from concourse.bass import _add_dep_helper
                _add_dep_helper(sqrt_inst.ins, sq_insts[-1].ins, False,
                                "keep ACT FIFO clear of the sqrt wait")
                r0 = sb.tile([128, nch], F32, name=f"r0{h}", tag=f"r0{h}")
                nc.vector.reciprocal(r0[:], y0[:])
                t0 = sb.tile([128, nch], F32, name=f"t0{h}", tag=f"t0{h}")
                nc.vector.tensor_tensor(out=t0[:], in0=n2g[:], in1=r0[:],
                                        op=mybir.AluOpType.mult)
                y1 = sb.tile([128, nch], F32, name=f"y1{h}", tag=f"y1{h}")
                nc.vector.tensor_tensor(out=y1[:], in0=y0[:], in1=t0[:],
                                        op=mybir.AluOpType.add)
                nc.vector.tensor_scalar(out=y1[:], in0=y1[:], scalar1=0.5,
                                        scalar2=EPS, op0=mybir.AluOpType.mult,
                                        op1=mybir.AluOpType.add)
                r1 = sb.tile([128, nch], F32, name=f"r1{h}", tag=f"r1{h}")
                nc.vector.reciprocal(r1[:], y1[:])
                t2 = sb.tile([128, nch], F32, name=f"t2{h}", tag=f"t2{h}")
                nc.vector.tensor_tensor(out=t2[:], in0=y1[:], in1=r1[:],
                                        op=mybir.AluOpType.mult)
                u = sb.tile([128, nch], F32, name=f"u{h}", tag=f"u{h}")
                nc.vector.tensor_scalar(out=u[:], in0=t2[:], scalar1=-1.0,
                                        scalar2=2.0, op0=mybir.AluOpType.mult,
                                        op1=mybir.AluOpType.add)
                r2 = sb.tile([128, nch], F32, name=f"r2{h}", tag=f"r2{h}")
                nc.vector.tensor_tensor(out=r2[:], in0=r1[:], in1=u[:],
                                        op=mybir.AluOpType.mult)
                s_sb = sb.tile([128, nch], F32, name=f"s{h}", tag=f"s{h}")
                nc.vector.tensor_tensor(out=s_sb[:],
                                        in0=m_sb[:, lo:lo + nch],
                                        in1=r2[:], op=mybir.AluOpType.mult)

                # xs = x * s (in place), then this half's output matmuls
                for j, c in enumerate(chunks):
                    xsl = slice(c * TOK, (c + 1) * TOK)
                    nc.vector.tensor_scalar_mul(
                        x_all[:, xsl], x_all[:, xsl], s_sb[:, j:j + 1])
                if h < len(HALVES) - 1:
                    for c in chunks:
                        for m in range(2):
                            lhs = x_all[:, c * TOK + m * 128:
                                        c * TOK + (m + 1) * 128]
                            for n in range(3):
                                nsl = slice(N_OFFS[n],
                                            N_OFFS[n] + N_TILES[n])
                                nc.tensor.matmul(
                                    psum_out[m, n][:], lhs,
                                    wc_tiles[c][:, nsl],
                                    start=False, stop=False,
                                )
                else:
                    # n-outer so each psum accumulator finishes early and
                    # its drain overlaps the remaining matmuls
                    for n in range(3):
                        nsl = slice(N_OFFS[n], N_OFFS[n] + N_TILES[n])
                        for c in chunks:
                            last = c == NCHUNK - 1
                            for m in range(2):
                                lhs = x_all[:, c * TOK + m * 128:
                                            c * TOK + (m + 1) * 128]
                                nc.tensor.matmul(
                                    psum_out[m, n][:], lhs,
                                    wc_tiles[c][:, nsl],
                                    start=False, stop=last,
                                )
                        for m in range(2):
                            st = stp.tile([128, N_TILES[n]], F32,
                                          name=f"st{m}{n}", tag="st")
                            nc.vector.tensor_copy(st[:], psum_out[m, n][:])
                            nc.sync.dma_start(
                                out[m * 128:(m + 1) * 128, nsl], st[:])

            parts_h = {}
            for h, chunks in enumerate(HALVES):
                parts_h[h] = sb.tile([128, len(chunks)], F32,
                                     name=f"parts{h}", tag=f"parts{h}")
                for j, c in enumerate(chunks):
                    phase1(c, parts_h[h], j)
            for h, chunks in enumerate(HALVES):
                phase23(h, chunks, parts_h[h])

    nc.compile()
    return nc


def _prep_inputs(x, weight, lora_A, lora_B, lora_M, bias):
    """Shard + lay out the full inputs for the 8 cores (host-side data
    marshaling only)."""
    x = np.asarray(x, np.float32)
    weight = np.asarray(weight, np.float32)
    lora_A = np.asarray(lora_A, np.float32)
    lora_B = np.asarray(lora_B, np.float32)
    lora_M = np.asarray(lora_M, np.float32)
    bias = np.asarray(bias, np.float32)

    xt = np.ascontiguousarray(x.reshape(TOK, IN).T.astype(np.float16))
    a_packed = np.empty((128, IN // 2), np.float16)
    a_packed[0:64] = lora_A[:, 0:IN // 2]
    a_packed[64:128] = lora_A[:, IN // 2:]
    mt = np.ascontiguousarray(lora_M.reshape(NCHUNK, 128).T)     # [128, 32]
    ones = np.ones((1, 128), np.float16)

    in_maps = []
    for c in range(NCORES):
        sl = slice(c * OSH, (c + 1) * OSH)
        b2t_half = (SCALING * lora_B[sl, :]).T.astype(np.float16)
        in_maps.append(dict(
            wt=np.ascontiguousarray(weight[sl, :].T),            # [IN, OSH] f32
            xt=xt,
            ap=a_packed,
            b2t=np.ascontiguousarray(np.concatenate([b2t_half] * 2, axis=0)),
            bias=np.ascontiguousarray(
                bias[sl].reshape(1, OSH).astype(np.float16)),
            ones=ones,
            mt=mt,
        ))
    return in_maps


def _run(inputs, trace=False):
    nc = _build()
    in_maps = _prep_inputs(**inputs)
    res = run_bass_kernel_spmd(nc, in_maps, core_ids=list(range(NCORES)),
                               trace=trace)
    full = np.concatenate([res.results[c]["out"] for c in range(NCORES)],
                          axis=1)
    return full.reshape(B, S, OUT), res


def kernel(x, weight, lora_A, lora_B, lora_M, bias):
    out, _ = _run(dict(x=x, weight=weight, lora_A=lora_A, lora_B=lora_B,
                       lora_M=lora_M, bias=bias))
    return out


def kernel_profiled(**inputs):
    """Like kernel() but with NTFF tracing; returns (out, exec_time_ns)."""
    out, res = _run(inputs, trace=True)
    return out, res.exec_time_ns


# revision 20
# speedup vs baseline: 1.0542x; 1.0537x over previous
"""Trainium2 Bass kernel for nn_LoraLinear (DoRA-style LoraLinear forward).

Reference computation:
    Wc   = weight + 2.0 * (lora_B @ lora_A)            # [OUT, IN]
    norm = ||Wc||_2 over OUT axis + 1e-6               # [1, IN]
    out  = x @ (lora_M * Wc / norm).T + bias           # [B, S, OUT]

Key algebraic identity: (lora_M * Wc / norm).T applied to x equals
(x * s) @ Wc.T with s = lora_M / norm, so W_eff is never materialized.

Sharding (8 NeuronCores, tensor/column parallel):
    - OUT = 11008 = 8 * 1376 -> each core owns 1376 output columns of
      weight / lora_B / bias.
    - x, lora_A, lora_M replicated.
    - Each core computes partial column sums-of-squares of Wc over its
      OUT shard; two AllReduces (one per half of the IN dimension)
      combine the 8 partials, so the first collective and its dependent
      matmuls overlap the tail of the weight stream.

Device layout: IN lives on partitions everywhere. The host passes W and
x pre-transposed so tiles land as [IN-part, OUT-free] / [IN-part, tok],
which is what both the TensorEngine matmuls and the free-dim norm
reduction want. W streams in fp32 (norms in full precision); the
resident combined weight + activations are fp16 for full-rate matmuls.
"""

import functools

import numpy as np

import concourse.tile as tile
from concourse import bacc, mybir
from concourse.bass_utils import run_bass_kernel_spmd

F32 = mybir.dt.float32
F16 = mybir.dt.float16

NCORES = 8
B, S, IN, OUT, R = 8, 32, 4096, 11008, 64
TOK = B * S                      # 256
OSH = OUT // NCORES              # 1376 per-core output shard
NCHUNK = IN // 128               # 32 IN chunks of 128
N_TILES = (512, 512, 352)        # OUT-shard tiling for psum (<=512 fp32)
N_OFFS = (0, 512, 1024)
SCALING = 2.0
EPS = 1e-6
HALVES = (tuple(range(0, 16)), tuple(range(16, 32)))
N_WARM = 16


@functools.lru_cache(maxsize=1)
def _build():
    nc = bacc.Bacc("TRN2", target_bir_lowering=False, debug=False,
                   num_devices=NCORES)

    wt = nc.dram_tensor("wt", [IN, OSH], F32, kind="ExternalInput").ap()
    xt = nc.dram_tensor("xt", [IN, TOK], F16, kind="ExternalInput").ap()
    ap_ = nc.dram_tensor("ap", [128, IN // 2], F16, kind="ExternalInput").ap()
    b2t = nc.dram_tensor("b2t", [2 * R, OSH], F16, kind="ExternalInput").ap()
    bias = nc.dram_tensor("bias", [1, OSH], F16, kind="ExternalInput").ap()
    ones = nc.dram_tensor("ones", [1, 128], F16, kind="ExternalInput").ap()
    mt = nc.dram_tensor("mt", [128, NCHUNK], F32, kind="ExternalInput").ap()

    out = nc.dram_tensor("out", [TOK, OSH], F32, kind="ExternalOutput").ap()

    with tile.TileContext(nc) as tc:
        with (
            tc.tile_pool(name="wstream", bufs=8) as wsp,
            tc.tile_pool(name="wc", bufs=1) as wcp,
            tc.tile_pool(name="sb", bufs=1) as sb,
            tc.tile_pool(name="stage", bufs=2) as stp,
            tc.tile_pool(name="ps", bufs=2, space="PSUM") as psp,
            tc.tile_pool(name="pso", bufs=1, space="PSUM") as pso,
            tc.tile_pool(name="dram", bufs=1, space="DRAM") as dram,
        ):
            # ---- constants / small tensors ----
            a_sb = sb.tile([128, IN // 2], F16, name="a_sb")
            b2t_sb = sb.tile([2 * R, OSH], F16, name="b2t_sb")
            bias_sb = sb.tile([1, OSH], F16, name="bias_sb")
            ones_sb = sb.tile([1, 128], F16, name="ones_sb")
            m_sb = sb.tile([128, NCHUNK], F32, name="m_sb")
            x_all = sb.tile([128, NCHUNK * TOK], F16, name="x_all")
            warm_src = sb.tile([128, 512], F32, name="warm_src")
            nc.vector.memset(warm_src[:], 0.001)
            sq_scr = sb.tile([128, OSH], F16, name="sq_scr")

            nc.sync.dma_start(a_sb[:], ap_)
            nc.sync.dma_start(b2t_sb[:], b2t)
            nc.sync.dma_start(bias_sb[:], bias)
            nc.sync.dma_start(ones_sb[:], ones)
            nc.sync.dma_start(m_sb[:], mt)

            # ---- persistent psum accumulators, seeded with the bias ----
            psum_out = {}
            for m in range(2):
                for n in range(3):
                    psum_out[m, n] = pso.tile(
                        [128, N_TILES[n]], F32, name=f"po{m}{n}", tag=f"po{m}{n}"
                    )
                    nc.tensor.matmul(
                        psum_out[m, n][:],
                        ones_sb[0:1, 0:128],
                        bias_sb[0:1, N_OFFS[n]:N_OFFS[n] + N_TILES[n]],
                        start=True, stop=False,
                    )

            wc_tiles = {}
            sq_insts = []

            def phase1(c, parts, jcol):
                """Stream W chunk c, build resident fp16 Wc, norm partial."""
                wst = wsp.tile([128, OSH], F32, name=f"ws{c}", tag="ws")
                nc.sync.dma_start(wst[:], wt[c * 128:(c + 1) * 128, :])
                wct = wcp.tile([128, OSH], F16, name=f"wc{c}", tag=f"wc{c}")
                wc_tiles[c] = wct
                if c < NCHUNK // 2:
                    a_chunk = a_sb[0:64, c * 128:(c + 1) * 128]
                    rlo = 0
                else:
                    cc = c - NCHUNK // 2
                    a_chunk = a_sb[64:128, cc * 128:(cc + 1) * 128]
                    rlo = 64
                for n in range(3):
                    nsl = slice(N_OFFS[n], N_OFFS[n] + N_TILES[n])
                    pl = psp.tile([128, 512], F32, name=f"pl{c}{n}", tag="pl")
                    nc.tensor.matmul(
                        pl[:, 0:N_TILES[n]], a_chunk,
                        b2t_sb[rlo:rlo + 64, nsl],
                        start=True, stop=True,
                    )
                    # Wc(fp16) = W(fp32) + 2*B@A(psum fp32)
                    nc.vector.tensor_tensor(
                        out=wct[:, nsl], in0=wst[:, nsl],
                        in1=pl[:, 0:N_TILES[n]], op=mybir.AluOpType.add,
                    )
                # one wide column-sum-of-squares per chunk on ACT; the
                # squared values go to a scratch tile and are discarded
                sq_insts.append(nc.scalar.activation(
                    sq_scr[:], wct[:],
                    mybir.ActivationFunctionType.Square,
                    accum_out=parts[:, jcol:jcol + 1],
                ))
                # prefetch x chunk (fp16) into the big resident x buffer
                # (gpsimd SWDGE ring: keeps the sync ring pure W stream)
                nc.gpsimd.dma_start(
                    x_all[:, c * TOK:(c + 1) * TOK],
                    xt[c * 128:(c + 1) * 128, :],
                )

            def phase23(h, chunks, parts):
                """AllReduce this half's partials, compute s, warm the PE,
                then run this half's output matmuls."""
                nch = len(chunks)
                lo = chunks[0]
                cin = dram.tile([128, nch], F32, name=f"ci{h}", tag=f"ci{h}")
                cout = dram.tile([128, nch], F32, name=f"co{h}", tag=f"co{h}",
                                 addr_space="Shared")
                nc.gpsimd.dma_start(cin[:], parts[:])
                nc.gpsimd.collective_compute(
                    "AllReduce", mybir.AluOpType.add,
                    replica_groups=[list(range(NCORES))],
                    ins=[cin[:].opt()], outs=[cout[:].opt()],
                )
                n2g = sb.tile([128, nch], F32, name=f"n2g{h}", tag=f"n2g{h}")
                nc.sync.dma_start(n2g[:], cout[:])

                # PE warmup: junk matmuls gated on this half's partials so
                # the PE clock stays hot across the collective window
                wps = stp.tile([16, 32], F32, name=f"wps{h}", tag="wps")
                for wi in range(N_WARM):
                    plw = psp.tile([128, 512], F32, name=f"plw{h}{wi}",
                                   tag="pl")
                    nc.tensor.matmul(plw[0:nch, :], parts[:],
                                     warm_src[:],
                                     start=True, stop=True)
                    if wi == N_WARM - 1:
                        nc.vector.tensor_copy(wps[:], plw[0:16, 0:32])

                # s = m/(sqrt(n2g)+eps): ACT sqrt + Newton, DVE recip + Newton
                y0 = sb.tile([128, nch], F32, name=f"y0{h}", tag=f"y0{h}")
                sqrt_inst = nc.scalar.activation(
                    y0[:], n2g[:], mybir.ActivationFunctionType.Sqrt)
                from concourse.bass import _add_dep_helper
                _add_dep_helper(sqrt_inst.ins, sq_insts[-1].ins, False,
                                "keep ACT FIFO clear of the sqrt wait")
                r0 = sb.tile([128, nch], F32, name=f"r0{h}", tag=f"r0{h}")
                nc.vector.reciprocal(r0[:], y0[:])
                t0 = sb.tile([128, nch], F32, name=f"t0{h}", tag=f"t0{h}")
                nc.vector.tensor_tensor(out=t0[:], in0=n2g[:], in1=r0[:],
                                        op=mybir.AluOpType.mult)
                y1 = sb.tile([128, nch], F32, name=f"y1{h}", tag=f"y1{h}")
                nc.vector.tensor_tensor(out=y1[:], in0=y0[:], in1=t0[:],
                                        op=mybir.AluOpType.add)
                nc.vector.tensor_scalar(out=y1[:], in0=y1[:], scalar1=0.5,
                                        scalar2=EPS, op0=mybir.AluOpType.mult,
                                        op1=mybir.AluOpType.add)
                r1 = sb.tile([128, nch], F32, name=f"r1{h}", tag=f"r1{h}")
                nc.vector.reciprocal(r1[:], y1[:])
                s_sb = sb.tile([128, nch], F32, name=f"s{h}", tag=f"s{h}")
                nc.vector.tensor_tensor(out=s_sb[:],
                                        in0=m_sb[:, lo:lo + nch],
                                        in1=r1[:], op=mybir.AluOpType.mult)

                # xs = x * s (in place), then this half's output matmuls
                for j, c in enumerate(chunks):
                    xsl = slice(c * TOK, (c + 1) * TOK)
                    nc.vector.tensor_scalar_mul(
                        x_all[:, xsl], x_all[:, xsl], s_sb[:, j:j + 1])
                if h < len(HALVES) - 1:
                    for c in chunks:
                        for m in range(2):
                            lhs = x_all[:, c * TOK + m * 128:
                                        c * TOK + (m + 1) * 128]
                            for n in range(3):
                                nsl = slice(N_OFFS[n],
                                            N_OFFS[n] + N_TILES[n])
                                nc.tensor.matmul(
                                    psum_out[m, n][:], lhs,
                                    wc_tiles[c][:, nsl],
                                    start=False, stop=False,
                                )
                else:
                    # n-outer so each psum accumulator finishes early and
                    # its drain overlaps the remaining matmuls
                    for n in range(3):
                        nsl = slice(N_OFFS[n], N_OFFS[n] + N_TILES[n])
                        for c in chunks:
                            last = c == NCHUNK - 1
                            for m in range(2):
                                lhs = x_all[:, c * TOK + m * 128:
                                            c * TOK + (m + 1) * 128]
                                nc.tensor.matmul(
                                    psum_out[m, n][:], lhs,
                                    wc_tiles[c][:, nsl],
                                    start=False, stop=last,
                                )
                        for m in range(2):
                            st = stp.tile([128, N_TILES[n]], F32,
                                          name=f"st{m}{n}", tag="st")
                            nc.vector.tensor_copy(st[:], psum_out[m, n][:])
                            nc.sync.dma_start(
                                out[m * 128:(m + 1) * 128, nsl], st[:])

            parts_h = {}
            for h, chunks in enumerate(HALVES):
                parts_h[h] = sb.tile([128, len(chunks)], F32,
                                     name=f"parts{h}", tag=f"parts{h}")
                for j, c in enumerate(chunks):
                    phase1(c, parts_h[h], j)
            for h, chunks in enumerate(HALVES):
                phase23(h, chunks, parts_h[h])

    nc.compile()
    return nc


def _prep_inputs(x, weight, lora_A, lora_B, lora_M, bias):
    """Shard + lay out the full inputs for the 8 cores (host-side data
    marshaling only)."""
    x = np.asarray(x, np.float32)
    weight = np.asarray(weight, np.float32)
    lora_A = np.asarray(lora_A, np.float32)
    lora_B = np.asarray(lora_B, np.float32)
    lora_M = np.asarray(lora_M, np.float32)
    bias = np.asarray(bias, np.float32)

    xt = np.ascontiguousarray(x.reshape(TOK, IN).T.astype(np.float16))
    a_packed = np.empty((128, IN // 2), np.float16)
    a_packed[0:64] = lora_A[:, 0:IN // 2]
    a_packed[64:128] = lora_A[:, IN // 2:]
    mt = np.ascontiguousarray(lora_M.reshape(NCHUNK, 128).T)     # [128, 32]
    ones = np.ones((1, 128), np.float16)

    in_maps = []
    for c in range(NCORES):
        sl = slice(c * OSH, (c + 1) * OSH)
        b2t_half = (SCALING * lora_B[sl, :]).T.astype(np.float16)
        in_maps.append(dict(
            wt=np.ascontiguousarray(weight[sl, :].T),            # [IN, OSH] f32
            xt=xt,
            ap=a_packed,
            b2t=np.ascontiguousarray(np.concatenate([b2t_half] * 2, axis=0)),
            bias=np.ascontiguousarray(
                bias[sl].reshape(1, OSH).astype(np.float16)),
            ones=ones,
            mt=mt,
        ))
    return in_maps


def _run(inputs, trace=False):
    nc = _build()
    in_maps = _prep_inputs(**inputs)
    res = run_bass_kernel_spmd(nc, in_maps, core_ids=list(range(NCORES)),
                               trace=trace)
    full = np.concatenate([res.results[c]["out"] for c in range(NCORES)],
                          axis=1)
    return full.reshape(B, S, OUT), res


def kernel(x, weight, lora_A, lora_B, lora_M, bias):
    out, _ = _run(dict(x=x, weight=weight, lora_A=lora_A, lora_B=lora_B,
                       lora_M=lora_M, bias=bias))
    return out


def kernel_profiled(**inputs):
    """Like kernel() but with NTFF tracing; returns (out, exec_time_ns)."""
    out, res = _run(inputs, trace=True)
    return out, res.exec_time_ns


# revision 21
# speedup vs baseline: 1.1690x; 1.1089x over previous
"""Trainium2 Bass kernel for nn_LoraLinear (DoRA-style LoraLinear forward).

Reference computation:
    Wc   = weight + 2.0 * (lora_B @ lora_A)            # [OUT, IN]
    norm = ||Wc||_2 over OUT axis + 1e-6               # [1, IN]
    out  = x @ (lora_M * Wc / norm).T + bias           # [B, S, OUT]

Key algebraic identity: (lora_M * Wc / norm).T applied to x equals
(x * s) @ Wc.T with s = lora_M / norm, so W_eff is never materialized.

Sharding (8 NeuronCores, tensor/column parallel):
    - OUT = 11008 = 8 * 1376 -> each core owns 1376 output columns of
      weight / lora_B / bias.
    - x, lora_A, lora_M replicated.
    - Each core computes partial column sums-of-squares of Wc over its
      OUT shard; two AllReduces (one per half of the IN dimension)
      combine the 8 partials, so the first collective and its dependent
      matmuls overlap the tail of the weight stream.

Device layout: IN lives on partitions everywhere. The host passes W and
x pre-transposed so tiles land as [IN-part, OUT-free] / [IN-part, tok],
which is what both the TensorEngine matmuls and the free-dim norm
reduction want. W streams in fp32 (norms in full precision); the
resident combined weight + activations are fp16 for full-rate matmuls.
"""

import functools

import numpy as np

import concourse.tile as tile
from concourse import bacc, mybir
from concourse.bass_utils import run_bass_kernel_spmd

F32 = mybir.dt.float32
F16 = mybir.dt.float16

NCORES = 8
B, S, IN, OUT, R = 8, 32, 4096, 11008, 64
TOK = B * S                      # 256
OSH = OUT // NCORES              # 1376 per-core output shard
NCHUNK = IN // 128               # 32 IN chunks of 128
N_TILES = (512, 512, 352)        # OUT-shard tiling for psum (<=512 fp32)
N_OFFS = (0, 512, 1024)
SCALING = 2.0
EPS = 1e-6
HALVES = (tuple(range(0, 16)), tuple(range(16, 32)))
N_WARM = 16


@functools.lru_cache(maxsize=1)
def _build():
    nc = bacc.Bacc("TRN2", target_bir_lowering=False, debug=False,
                   num_devices=NCORES)

    wt = nc.dram_tensor("wt", [IN, OSH], F32, kind="ExternalInput").ap()
    xt = nc.dram_tensor("xt", [IN, TOK], F16, kind="ExternalInput").ap()
    ap_ = nc.dram_tensor("ap", [128, IN // 2], F16, kind="ExternalInput").ap()
    b2t = nc.dram_tensor("b2t", [2 * R, OSH], F16, kind="ExternalInput").ap()
    bias = nc.dram_tensor("bias", [1, OSH], F16, kind="ExternalInput").ap()
    ones = nc.dram_tensor("ones", [1, 128], F16, kind="ExternalInput").ap()
    mt = nc.dram_tensor("mt", [128, NCHUNK], F32, kind="ExternalInput").ap()

    out = nc.dram_tensor("out", [TOK, OSH], F32, kind="ExternalOutput").ap()

    with tile.TileContext(nc) as tc:
        with (
            tc.tile_pool(name="wstream", bufs=8) as wsp,
            tc.tile_pool(name="wc", bufs=1) as wcp,
            tc.tile_pool(name="sb", bufs=1) as sb,
            tc.tile_pool(name="stage", bufs=2) as stp,
            tc.tile_pool(name="ps", bufs=2, space="PSUM") as psp,
            tc.tile_pool(name="pso", bufs=1, space="PSUM") as pso,
            tc.tile_pool(name="dram", bufs=1, space="DRAM") as dram,
        ):
            # ---- constants / small tensors ----
            a_sb = sb.tile([128, IN // 2], F16, name="a_sb")
            b2t_sb = sb.tile([2 * R, OSH], F16, name="b2t_sb")
            bias_sb = sb.tile([1, OSH], F16, name="bias_sb")
            ones_sb = sb.tile([1, 128], F16, name="ones_sb")
            m_sb = sb.tile([128, NCHUNK], F32, name="m_sb")
            x_all = sb.tile([128, NCHUNK * TOK], F16, name="x_all")
            warm_src = sb.tile([128, 512], F32, name="warm_src")
            nc.vector.memset(warm_src[:], 0.001)
            sq_scr = sb.tile([128, OSH], F16, name="sq_scr")

            nc.sync.dma_start(a_sb[:], ap_)
            nc.sync.dma_start(b2t_sb[:], b2t)
            nc.sync.dma_start(bias_sb[:], bias)
            nc.sync.dma_start(ones_sb[:], ones)
            nc.sync.dma_start(m_sb[:], mt)

            # ---- persistent psum accumulators, seeded with the bias ----
            psum_out = {}
            for m in range(2):
                for n in range(3):
                    psum_out[m, n] = pso.tile(
                        [128, N_TILES[n]], F32, name=f"po{m}{n}", tag=f"po{m}{n}"
                    )
                    nc.tensor.matmul(
                        psum_out[m, n][:],
                        ones_sb[0:1, 0:128],
                        bias_sb[0:1, N_OFFS[n]:N_OFFS[n] + N_TILES[n]],
                        start=True, stop=False,
                    )

            wc_tiles = {}
            sq_insts = []

            def phase1(c, parts, jcol):
                """Stream W chunk c, build resident fp16 Wc, norm partial."""
                wst = wsp.tile([128, OSH], F32, name=f"ws{c}", tag="ws")
                nc.sync.dma_start(wst[:], wt[c * 128:(c + 1) * 128, :])
                wct = wcp.tile([128, OSH], F16, name=f"wc{c}", tag=f"wc{c}")
                wc_tiles[c] = wct
                if c < NCHUNK // 2:
                    a_chunk = a_sb[0:64, c * 128:(c + 1) * 128]
                    rlo = 0
                else:
                    cc = c - NCHUNK // 2
                    a_chunk = a_sb[64:128, cc * 128:(cc + 1) * 128]
                    rlo = 64
                for n in range(3):
                    nsl = slice(N_OFFS[n], N_OFFS[n] + N_TILES[n])
                    pl = psp.tile([128, 512], F32, name=f"pl{c}{n}", tag="pl")
                    nc.tensor.matmul(
                        pl[:, 0:N_TILES[n]], a_chunk,
                        b2t_sb[rlo:rlo + 64, nsl],
                        start=True, stop=True,
                    )
                    # Wc(fp16) = W(fp32) + 2*B@A(psum fp32)
                    nc.vector.tensor_tensor(
                        out=wct[:, nsl], in0=wst[:, nsl],
                        in1=pl[:, 0:N_TILES[n]], op=mybir.AluOpType.add,
                    )
                # one wide column-sum-of-squares per chunk on ACT; the
                # squared values go to a scratch tile and are discarded
                sq_insts.append(nc.scalar.activation(
                    sq_scr[:], wct[:],
                    mybir.ActivationFunctionType.Square,
                    accum_out=parts[:, jcol:jcol + 1],
                ))
                # prefetch x chunk (fp16) into the big resident x buffer
                # (gpsimd SWDGE ring: keeps the sync ring pure W stream)
                nc.gpsimd.dma_start(
                    x_all[:, c * TOK:(c + 1) * TOK],
                    xt[c * 128:(c + 1) * 128, :],
                )

            def phase23(h, chunks, parts):
                """AllReduce this half's partials, compute s, warm the PE,
                then run this half's output matmuls."""
                nch = len(chunks)
                lo = chunks[0]
                cin = dram.tile([128, nch], F32, name=f"ci{h}", tag=f"ci{h}")
                cout = dram.tile([128, nch], F32, name=f"co{h}", tag=f"co{h}",
                                 addr_space="Shared")
                nc.gpsimd.dma_start(cin[:], parts[:])
                nc.gpsimd.collective_compute(
                    "AllReduce", mybir.AluOpType.add,
                    replica_groups=[list(range(NCORES))],
                    ins=[cin[:].opt()], outs=[cout[:].opt()],
                )
                n2g = sb.tile([128, nch], F32, name=f"n2g{h}", tag=f"n2g{h}")
                nc.sync.dma_start(n2g[:], cout[:])

                # PE warmup: junk matmuls gated on this half's partials so
                # the PE clock stays hot across the collective window.
                # Only for the first half — by the second the PE is already
                # saturated with real matmuls and junk would delay them.
                wps = stp.tile([16, 32], F32, name=f"wps{h}", tag="wps")
                for wi in range(N_WARM if h == 0 else 0):
                    plw = psp.tile([128, 512], F32, name=f"plw{h}{wi}",
                                   tag="pl")
                    nc.tensor.matmul(plw[0:nch, :], parts[:],
                                     warm_src[:],
                                     start=True, stop=True)
                    if wi == N_WARM - 1:
                        nc.vector.tensor_copy(wps[:], plw[0:16, 0:32])

                # s = m/(sqrt(n2g)+eps): ACT sqrt + Newton, DVE recip + Newton
                y0 = sb.tile([128, nch], F32, name=f"y0{h}", tag=f"y0{h}")
                sqrt_inst = nc.scalar.activation(
                    y0[:], n2g[:], mybir.ActivationFunctionType.Sqrt)
                from concourse.bass import _add_dep_helper
                _add_dep_helper(sqrt_inst.ins, sq_insts[-1].ins, False,
                                "keep ACT FIFO clear of the sqrt wait")
                r0 = sb.tile([128, nch], F32, name=f"r0{h}", tag=f"r0{h}")
                nc.vector.reciprocal(r0[:], y0[:])
                t0 = sb.tile([128, nch], F32, name=f"t0{h}", tag=f"t0{h}")
                nc.vector.tensor_tensor(out=t0[:], in0=n2g[:], in1=r0[:],
                                        op=mybir.AluOpType.mult)
                y1 = sb.tile([128, nch], F32, name=f"y1{h}", tag=f"y1{h}")
                nc.vector.tensor_tensor(out=y1[:], in0=y0[:], in1=t0[:],
                                        op=mybir.AluOpType.add)
                nc.vector.tensor_scalar(out=y1[:], in0=y1[:], scalar1=0.5,
                                        scalar2=EPS, op0=mybir.AluOpType.mult,
                                        op1=mybir.AluOpType.add)
                r1 = sb.tile([128, nch], F32, name=f"r1{h}", tag=f"r1{h}")
                nc.vector.reciprocal(r1[:], y1[:])
                s_sb = sb.tile([128, nch], F32, name=f"s{h}", tag=f"s{h}")
                nc.vector.tensor_tensor(out=s_sb[:],
                                        in0=m_sb[:, lo:lo + nch],
                                        in1=r1[:], op=mybir.AluOpType.mult)

                # xs = x * s (in place), then this half's output matmuls
                for j, c in enumerate(chunks):
                    xsl = slice(c * TOK, (c + 1) * TOK)
                    nc.vector.tensor_scalar_mul(
                        x_all[:, xsl], x_all[:, xsl], s_sb[:, j:j + 1])
                if h < len(HALVES) - 1:
                    for c in chunks:
                        for m in range(2):
                            lhs = x_all[:, c * TOK + m * 128:
                                        c * TOK + (m + 1) * 128]
                            for n in range(3):
                                nsl = slice(N_OFFS[n],
                                            N_OFFS[n] + N_TILES[n])
                                nc.tensor.matmul(
                                    psum_out[m, n][:], lhs,
                                    wc_tiles[c][:, nsl],
                                    start=False, stop=False,
                                )
                else:
                    # n-outer so each psum accumulator finishes early and
                    # its drain overlaps the remaining matmuls
                    for n in range(3):
                        nsl = slice(N_OFFS[n], N_OFFS[n] + N_TILES[n])
                        for c in chunks:
                            last = c == NCHUNK - 1
                            for m in range(2):
                                lhs = x_all[:, c * TOK + m * 128:
                                            c * TOK + (m + 1) * 128]
                                nc.tensor.matmul(
                                    psum_out[m, n][:], lhs,
                                    wc_tiles[c][:, nsl],
                                    start=False, stop=last,
                                )
                        for m in range(2):
                            st = stp.tile([128, N_TILES[n]], F32,
                                          name=f"st{m}{n}", tag="st")
                            nc.vector.tensor_copy(st[:], psum_out[m, n][:])
                            nc.sync.dma_start(
                                out[m * 128:(m + 1) * 128, nsl], st[:])

            parts_h = {}
            for h, chunks in enumerate(HALVES):
                parts_h[h] = sb.tile([128, len(chunks)], F32,
                                     name=f"parts{h}", tag=f"parts{h}")
                for j, c in enumerate(chunks):
                    phase1(c, parts_h[h], j)
            for h, chunks in enumerate(HALVES):
                phase23(h, chunks, parts_h[h])

    nc.compile()
    return nc


def _prep_inputs(x, weight, lora_A, lora_B, lora_M, bias):
    """Shard + lay out the full inputs for the 8 cores (host-side data
    marshaling only)."""
    x = np.asarray(x, np.float32)
    weight = np.asarray(weight, np.float32)
    lora_A = np.asarray(lora_A, np.float32)
    lora_B = np.asarray(lora_B, np.float32)
    lora_M = np.asarray(lora_M, np.float32)
    bias = np.asarray(bias, np.float32)

    xt = np.ascontiguousarray(x.reshape(TOK, IN).T.astype(np.float16))
    a_packed = np.empty((128, IN // 2), np.float16)
    a_packed[0:64] = lora_A[:, 0:IN // 2]
    a_packed[64:128] = lora_A[:, IN // 2:]
    mt = np.ascontiguousarray(lora_M.reshape(NCHUNK, 128).T)     # [128, 32]
    ones = np.ones((1, 128), np.float16)

    in_maps = []
    for c in range(NCORES):
        sl = slice(c * OSH, (c + 1) * OSH)
        b2t_half = (SCALING * lora_B[sl, :]).T.astype(np.float16)
        in_maps.append(dict(
            wt=np.ascontiguousarray(weight[sl, :].T),            # [IN, OSH] f32
            xt=xt,
            ap=a_packed,
            b2t=np.ascontiguousarray(np.concatenate([b2t_half] * 2, axis=0)),
            bias=np.ascontiguousarray(
                bias[sl].reshape(1, OSH).astype(np.float16)),
            ones=ones,
            mt=mt,
        ))
    return in_maps


def _run(inputs, trace=False):
    nc = _build()
    in_maps = _prep_inputs(**inputs)
    res = run_bass_kernel_spmd(nc, in_maps, core_ids=list(range(NCORES)),
                               trace=trace)
    full = np.concatenate([res.results[c]["out"] for c in range(NCORES)],
                          axis=1)
    return full.reshape(B, S, OUT), res


def kernel(x, weight, lora_A, lora_B, lora_M, bias):
    out, _ = _run(dict(x=x, weight=weight, lora_A=lora_A, lora_B=lora_B,
                       lora_M=lora_M, bias=bias))
    return out


def kernel_profiled(**inputs):
    """Like kernel() but with NTFF tracing; returns (out, exec_time_ns)."""
    out, res = _run(inputs, trace=True)
    return out, res.exec_time_ns
